# revision 10
# baseline (speedup 1.0000x reference)
"""Trainium2 Bass kernel for segmented-LoRA linear (nn_Linear_73959336837249).

Math: out = x @ W.T + scale_g * ((x_g @ A_g.T) @ B_g.T), where the 16384
tokens form 4 contiguous segments of 4096, one adapter per segment.

Strategy:
  * Fold the LoRA update into the base weight per adapter on the host:
        Weff_g = W + s_g * B_g @ A_g        (exact algebraic identity)
    so each token segment needs a single dense matmul x_g @ Weff_g.T.
  * Shard tokens across the 8 NeuronCores (2048 tokens/core); each core's
    token range lives entirely inside one adapter segment, so each core
    gets exactly one [2048, 2048] effective weight.
  * On device: one big [2048 x 2048] @ [2048 x 2048] matmul per core,
    K-tiled over PSUM. Inputs are fed as bf16 (rel err ~2e-3, well inside
    the 2e-2 gate); accumulation is fp32 in PSUM.

Schedule (v3, from perfetto trace analysis of v1/v2):
  * Steady state already runs at the warm N=512 floor (216 ns/MM); all
    recoverable time is in the first ~40 us: input-arrival stalls.
    Measured DMA behavior: transfers are sliced into 1 KB/partition
    packets round-robined over 16 engines; aggregate supply is
    ~300-350 GB/s, split roughly evenly between rings when two hwdge
    rings are active, with ~1-2 us completion latency per piece. A
    2-tile interleave demands 303 GB/s of W alone (v1) and a 4-tile
    one 300 GB/s of W+x (v2) — both stall.
  * v3 interleaves EIGHT token tiles (one PSUM bank each, all 8 banks)
    and sweeps the 4 o-chunks sequentially. First-use demand drops to
    ~224 GB/s (256 KB x + 131 KB W per 1.73 us k-step), under what a
    single ring supplies. All early pieces ride the SCALAR ring in
    exact consumption order (the proven v1 discipline — concurrent
    rings race each other for the shared packet engines).
  * W is laid out o-major in HBM ([o][k][512]) so stream order equals
    consumption order; x for tiles 0-7 is k-major ("xo": [k][t][128]).
  * Each sweep's last 4 k-steps run t-major so the 8 banks complete
    staggered ~0.86 us apart: the Vector copies (0.69 us each) keep up
    and the next sweep never waits on a bank.
  * Steady tiles t=8..14: k-outer / o-inner, 4+4 banks double-buffered.
  * Tail tile t=15 runs o-outer; the final bank's copy-out is split into
    2x256 cols on separate rings to shorten the last-DMA tail.

Self-contained: hardcodes all shapes; no file I/O.
"""

import numpy as np

# Problem shapes (hardcoded per contest contract)
N_ADAPTERS = 4
RANK = 16
D_IN = 2048
D_OUT = 2048
TOKENS = 16384
N_CORES = 8

T_LOC = TOKENS // N_CORES  # 2048 tokens per core
P = 128                    # partitions
KT = D_IN // P             # 16 contraction tiles
TT = T_LOC // P            # 16 token tiles per core
ON = 512                   # output-column tile (one PSUM bank of fp32)
NO = D_OUT // ON           # 4 o-tiles
NOCT = 8                   # token tiles interleaved in the early phase

N_WARM = 20                # N=128 warm-up matmuls before the first real matmul

IN_DTYPE = "bf16"

_NC = {}


def _np_in_dtype():
    import ml_dtypes

    return np.dtype(ml_dtypes.bfloat16)


def _build_nc():
    import concourse.mybir as mybir
    import concourse.tile as tile
    from concourse import bacc

    fp32 = mybir.dt.float32
    idt = mybir.dt.bfloat16

    nc = bacc.Bacc(None, target_bir_lowering=False)

    # xo[p, k*1024 + t*128 + j] = x_tok[t*128+j, k*128+p]  (tiles 0..7, k-major)
    xo = nc.dram_tensor("xo", [P, KT * NOCT * P], idt, kind="ExternalInput")
    # xt[t-8, p, k*128+j] = x_tok[t*128+j, k*128+p]  (tiles 8..15, tile-major)
    xt = nc.dram_tensor("xt", [TT - NOCT, P, KT * P], idt, kind="ExternalInput")
    # wt[p, (o*KT + k)*512 + c] = Weff.T[k*128+p, o*512+c]  (o-major)
    wt = nc.dram_tensor("wt", [P, NO * KT * ON], idt, kind="ExternalInput")
    out = nc.dram_tensor("out", [T_LOC, D_OUT], fp32, kind="ExternalOutput")

    def wo(k, o):  # start column of W slice (k, o) in the o-major layout
        return (o * KT + k) * ON

    XK = NOCT * P  # xo columns per k (1024)

    with tile.TileContext(nc) as tc:
        with (
            tc.tile_pool(name="wpool", bufs=1) as wpool,
            tc.tile_pool(name="xopool", bufs=1) as xopool,
            tc.tile_pool(name="xpool", bufs=3) as xpool,
            tc.tile_pool(name="opool", bufs=3) as opool,
            tc.tile_pool(name="oqpool", bufs=10) as oqpool,
            tc.tile_pool(name="spool", bufs=1) as spool,
            tc.tile_pool(name="pspool", bufs=1, space="PSUM") as pspool,
        ):
            # Scratch for PE warm-up. Memset on Vector: it reaches "main"
            # earliest and the DVE does bf16 SBUF memsets at 4x rate.
            # (A GpSimd memset here measured a 46 us regression — don't.)
            scr = spool.tile([P, P], idt, tag="scr", name="scratch")
            nc.vector.memset(scr[:], 0)

            wall = wpool.tile([P, NO * KT * ON], idt, tag="w", name="wall")
            xo_s = xopool.tile([P, KT * XK], idt, tag="xo", name="xo_s")

            # xo rides the SYNC ring (idle until the out-chunk DMAs at
            # ~40us) while W rides the SCALAR ring — each ring paced well
            # under its capacity, and the two streams never queue behind
            # each other. k0's xo is split so the first matmul's gate
            # (xo k0 t0-3 + W k0-1 = 390 KB) completes as early as
            # possible. W-o0 goes in k-pairs: in the o-major layout a
            # k-pair is 2 KB/partition-row contiguous, which the packet
            # engines move ~2x faster than 1 KB rows (measured).
            def wdma(ka, kb, o):
                a, b = wo(ka, o), wo(kb - 1, o) + ON
                nc.scalar.dma_start(wall[:, a:b], wt[:, a:b])

            def xodma(ka, kb):
                nc.sync.dma_start(
                    xo_s[:, ka * XK : kb * XK], xo[:, ka * XK : kb * XK]
                )

            nc.sync.dma_start(xo_s[:, : XK // 2], xo[:, : XK // 2])
            nc.sync.dma_start(xo_s[:, XK // 2 : XK], xo[:, XK // 2 : XK])
            for k in range(1, 8):
                xodma(k, k + 1)
            for k in range(8, 16, 2):
                xodma(k, k + 2)
            for k in range(0, 16, 2):
                wdma(k, k + 2, 0)
            for o in range(1, NO):
                for k in range(0, 16, 4):
                    wdma(k, k + 4, o)

            def bank(slot, name):
                return pspool.tile([P, ON], fp32, tag=f"ps{slot}", name=name)

            def xsl(k, t):
                a = k * XK + t * P
                return xo_s[:, a : a + P]

            # PE warm-up: garbage matmuls on zeroed scratch open the HAM
            # clock-gate (needs ~3.4us of busy) before real data arrives
            # (~2 us after the first two triggers complete).
            ps_oct = [bank(t, f"ps_a_{t}") for t in range(NOCT)]
            for _ in range(N_WARM):
                nc.tensor.matmul(
                    ps_oct[0][:, :P], scr[:, :P], scr[:, :P], start=True, stop=True
                )

            # ---- early phase: 8-tile interleave, 4 o-sweeps ----
            KSW = 12  # k-outer for k<KSW, t-major for the last 4 k-steps
            for o in range(NO):
                if o > 0:
                    ps_oct = [bank(t, f"ps_{o}_{t}") for t in range(NOCT)]
                for k in range(KSW):
                    for t in range(NOCT):
                        nc.tensor.matmul(
                            ps_oct[t][:],
                            xsl(k, t),
                            wall[:, wo(k, o) : wo(k, o) + ON],
                            start=(k == 0),
                            stop=False,
                        )
                for t in range(NOCT):
                    for k in range(KSW, KT):
                        nc.tensor.matmul(
                            ps_oct[t][:],
                            xsl(k, t),
                            wall[:, wo(k, o) : wo(k, o) + ON],
                            start=False,
                            stop=(k == KT - 1),
                        )
                    oq = oqpool.tile([P, ON], fp32, tag="oq", name=f"oq_{o}_{t}")
                    nc.vector.tensor_copy(oq[:], ps_oct[t][:])
                    nc.sync.dma_start(
                        out[t * P : (t + 1) * P, o * ON : (o + 1) * ON], oq[:]
                    )

            # ---- steady phase: t=8..14, one tile at a time ----
            x_tiles = {}
            for t in (8, 9, 10):
                x_tiles[t] = xpool.tile([P, KT * P], idt, tag="x", name=f"x_{t}")
                nc.scalar.dma_start(x_tiles[t][:], xt[t - NOCT])
            for t in range(NOCT, TT - 1):
                if t not in x_tiles:
                    x_tiles[t] = xpool.tile([P, KT * P], idt, tag="x", name=f"x_{t}")
                    nc.scalar.dma_start(x_tiles[t][:], xt[t - NOCT])
                s = t % 2
                pst = [bank(4 * s + o, f"ps_{t}_{o}") for o in range(NO)]
                for k in range(KT):
                    lhsT = x_tiles[t][:, k * P : (k + 1) * P]
                    for o in range(NO):
                        nc.tensor.matmul(
                            pst[o][:],
                            lhsT,
                            wall[:, wo(k, o) : wo(k, o) + ON],
                            start=(k == 0),
                            stop=(k == KT - 1),
                        )
                o_t = opool.tile([P, D_OUT], fp32, tag="o", name=f"o_{t}")
                for o in range(NO):
                    nc.vector.tensor_copy(o_t[:, o * ON : (o + 1) * ON], pst[o][:])
                nc.sync.dma_start(
                    out[t * P : (t + 1) * P, : D_OUT // 2], o_t[:, : D_OUT // 2]
                )
                nc.sync.dma_start(
                    out[t * P : (t + 1) * P, D_OUT // 2 :], o_t[:, D_OUT // 2 :]
                )

            # ---- tail tile t=15: o-outer so banks complete progressively;
            # the last bank's copy-out is split 2x256 to shorten the tail ----
            t = TT - 1
            x_tiles[t] = xpool.tile([P, KT * P], idt, tag="x", name=f"x_{t}")
            nc.scalar.dma_start(x_tiles[t][:], xt[t - NOCT])
            ps15 = [bank(4 + o, f"ps_15_{o}") for o in range(NO)]
            o_t = opool.tile([P, D_OUT], fp32, tag="o", name="o_15")
            out_eng = [nc.sync, nc.sync, nc.gpsimd]
            for o in range(NO):
                for k in range(KT):
                    nc.tensor.matmul(
                        ps15[o][:],
                        x_tiles[t][:, k * P : (k + 1) * P],
                        wall[:, wo(k, o) : wo(k, o) + ON],
                        start=(k == 0),
                        stop=(k == KT - 1),
                    )
                if o < NO - 1:
                    nc.vector.tensor_copy(o_t[:, o * ON : (o + 1) * ON], ps15[o][:])
                    out_eng[o].dma_start(
                        out[t * P : (t + 1) * P, o * ON : (o + 1) * ON],
                        o_t[:, o * ON : (o + 1) * ON],
                    )
                else:
                    h = ON // 2
                    a = o * ON
                    nc.vector.tensor_copy(o_t[:, a : a + h], ps15[o][:, :h])
                    nc.scalar.dma_start(
                        out[t * P : (t + 1) * P, a : a + h], o_t[:, a : a + h]
                    )
                    nc.vector.tensor_copy(o_t[:, a + h : a + ON], ps15[o][:, h:])
                    nc.gpsimd.dma_start(
                        out[t * P : (t + 1) * P, a + h : a + ON],
                        o_t[:, a + h : a + ON],
                    )

    nc.compile()
    return nc


def _get_nc():
    if IN_DTYPE not in _NC:
        _NC[IN_DTYPE] = _build_nc()
    return _NC[IN_DTYPE]


def _prep_inputs(inputs):
    x = np.ascontiguousarray(np.asarray(inputs["x"], dtype=np.float32))
    W = np.asarray(inputs["W"], dtype=np.float32)
    lora_a = np.asarray(inputs["lora_a"], dtype=np.float32)
    lora_b = np.asarray(inputs["lora_b"], dtype=np.float32)
    scalings = np.asarray(inputs["scalings"], dtype=np.float32)
    idt = _np_in_dtype()

    # Fold LoRA into the transposed effective weight per adapter:
    # Weff.T = W.T + s * A.T @ B.T  -> [d_in, d_out], laid out o-major:
    # wt[p, (o*KT + k)*512 + c] = Weff.T[k*128+p, o*512+c].
    wts = []
    for g in range(N_ADAPTERS):
        weff_t = W.T + scalings[g] * (lora_a[g].T @ lora_b[g].T)
        wts.append(
            np.ascontiguousarray(
                weff_t.reshape(KT, P, NO, ON).transpose(1, 2, 0, 3).astype(idt)
            ).reshape(P, NO * KT * ON)
        )

    in_maps = []
    for c in range(N_CORES):
        xs = x[c * T_LOC : (c + 1) * T_LOC]  # [2048 tok, 2048 d]
        # tiles 0-7, k-major octo layout: [t, j, k, p] -> [p, k, t, j]
        xol = np.ascontiguousarray(
            xs[: NOCT * P].reshape(NOCT, P, KT, P).transpose(3, 2, 0, 1).astype(idt)
        ).reshape(P, KT * NOCT * P)
        # tiles 8-15, tile-major: [t, j, k, p] -> [t, p, k, j]
        xtl = np.ascontiguousarray(
            xs[NOCT * P :]
            .reshape(TT - NOCT, P, KT, P)
            .transpose(0, 3, 2, 1)
            .astype(idt)
        ).reshape(TT - NOCT, P, KT * P)
        in_maps.append(
            {
                "xo": xol,
                "xt": xtl,
                "wt": wts[c * T_LOC // (TOKENS // N_ADAPTERS)],
            }
        )
    return in_maps


def _run(inputs, trace=False, **kwargs):
    from concourse.bass_utils import run_bass_kernel_spmd

    nc = _get_nc()
    in_maps = _prep_inputs(inputs)
    res = run_bass_kernel_spmd(
        nc, in_maps, core_ids=list(range(N_CORES)), trace=trace, **kwargs
    )
    out = np.concatenate([r["out"] for r in res.results], axis=0)
    return out, res


def kernel(**inputs):
    out, _ = _run(inputs, trace=False)
    return out


# revision 12
# speedup vs baseline: 1.0139x; 1.0139x over previous
"""Trainium2 Bass kernel for segmented-LoRA linear (nn_Linear_73959336837249).

Math: out = x @ W.T + scale_g * ((x_g @ A_g.T) @ B_g.T), where the 16384
tokens form 4 contiguous segments of 4096, one adapter per segment.

Strategy:
  * Fold the LoRA update into the base weight per adapter on the host:
        Weff_g = W + s_g * B_g @ A_g        (exact algebraic identity)
    so each token segment needs a single dense matmul x_g @ Weff_g.T.
  * Shard tokens across the 8 NeuronCores (2048 tokens/core); each core's
    token range lives entirely inside one adapter segment, so each core
    gets exactly one [2048, 2048] effective weight.
  * On device: one big [2048 x 2048] @ [2048 x 2048] matmul per core,
    K-tiled over PSUM. Inputs are fed as bf16 (rel err ~2e-3, well inside
    the 2e-2 gate); accumulation is fp32 in PSUM.

Schedule (v3, from perfetto trace analysis of v1/v2):
  * Steady state already runs at the warm N=512 floor (216 ns/MM); all
    recoverable time is in the first ~40 us: input-arrival stalls.
    Measured DMA behavior: transfers are sliced into 1 KB/partition
    packets round-robined over 16 engines; aggregate supply is
    ~300-350 GB/s, split roughly evenly between rings when two hwdge
    rings are active, with ~1-2 us completion latency per piece. A
    2-tile interleave demands 303 GB/s of W alone (v1) and a 4-tile
    one 300 GB/s of W+x (v2) — both stall.
  * v3 interleaves EIGHT token tiles (one PSUM bank each, all 8 banks)
    and sweeps the 4 o-chunks sequentially. First-use demand drops to
    ~224 GB/s (256 KB x + 131 KB W per 1.73 us k-step), under what a
    single ring supplies. All early pieces ride the SCALAR ring in
    exact consumption order (the proven v1 discipline — concurrent
    rings race each other for the shared packet engines).
  * W is laid out o-major in HBM ([o][k][512]) so stream order equals
    consumption order; x for tiles 0-7 is k-major ("xo": [k][t][128]).
  * Each sweep's last 4 k-steps run t-major so the 8 banks complete
    staggered ~0.86 us apart: the Vector copies (0.69 us each) keep up
    and the next sweep never waits on a bank.
  * Steady tiles t=8..14: k-outer / o-inner, 4+4 banks double-buffered.
  * Tail tile t=15 runs o-outer; the final bank's copy-out is split into
    2x256 cols on separate rings to shorten the last-DMA tail.

Self-contained: hardcodes all shapes; no file I/O.
"""

import numpy as np

# Problem shapes (hardcoded per contest contract)
N_ADAPTERS = 4
RANK = 16
D_IN = 2048
D_OUT = 2048
TOKENS = 16384
N_CORES = 8

T_LOC = TOKENS // N_CORES  # 2048 tokens per core
P = 128                    # partitions
KT = D_IN // P             # 16 contraction tiles
TT = T_LOC // P            # 16 token tiles per core
ON = 512                   # output-column tile (one PSUM bank of fp32)
NO = D_OUT // ON           # 4 o-tiles
NOCT = 8                   # token tiles interleaved in the early phase

N_WARM = 27                # N=128 warm-up matmuls before the first real matmul

IN_DTYPE = "bf16"

_NC = {}


def _np_in_dtype():
    import ml_dtypes

    return np.dtype(ml_dtypes.bfloat16)


def _build_nc():
    import concourse.mybir as mybir
    import concourse.tile as tile
    from concourse import bacc

    fp32 = mybir.dt.float32
    idt = mybir.dt.bfloat16

    nc = bacc.Bacc(None, target_bir_lowering=False)

    # xo[p, k*1024 + t*128 + j] = x_tok[t*128+j, k*128+p]  (tiles 0..7, k-major)
    xo = nc.dram_tensor("xo", [P, KT * NOCT * P], idt, kind="ExternalInput")
    # xt[t-8, p, k*128+j] = x_tok[t*128+j, k*128+p]  (tiles 8..15, tile-major)
    xt = nc.dram_tensor("xt", [TT - NOCT, P, KT * P], idt, kind="ExternalInput")
    # wt[p, (o*KT + k)*512 + c] = Weff.T[k*128+p, o*512+c]  (o-major)
    wt = nc.dram_tensor("wt", [P, NO * KT * ON], idt, kind="ExternalInput")
    out = nc.dram_tensor("out", [T_LOC, D_OUT], fp32, kind="ExternalOutput")

    def wo(k, o):  # start column of W slice (k, o) in the o-major layout
        return (o * KT + k) * ON

    XK = NOCT * P  # xo columns per k (1024)

    with tile.TileContext(nc) as tc:
        with (
            tc.tile_pool(name="wpool", bufs=1) as wpool,
            tc.tile_pool(name="xopool", bufs=1) as xopool,
            tc.tile_pool(name="xpool", bufs=3) as xpool,
            tc.tile_pool(name="opool", bufs=3) as opool,
            tc.tile_pool(name="oqpool", bufs=10) as oqpool,
            tc.tile_pool(name="spool", bufs=1) as spool,
            tc.tile_pool(name="pspool", bufs=1, space="PSUM") as pspool,
        ):
            # Scratch for PE warm-up. Memset on Vector: it reaches "main"
            # earliest and the DVE does bf16 SBUF memsets at 4x rate.
            # (A GpSimd memset here measured a 46 us regression — don't.)
            scr = spool.tile([P, P], idt, tag="scr", name="scratch")
            nc.vector.memset(scr[:], 0)

            wall = wpool.tile([P, NO * KT * ON], idt, tag="w", name="wall")
            xo_s = xopool.tile([P, KT * XK], idt, tag="xo", name="xo_s")

            # All early pieces on ONE ring (scalar) in exact consumption
            # order — two concurrent rings race each other for the shared
            # packet engines and the W stream loses (measured twice).
            # k0's xo is split so the first matmul's gate (xo k0 t0-3 +
            # W k0 = 259 KB) completes as early as possible. After k0, W
            # goes in k-pairs: in the o-major layout a k-pair is 2 KB/
            # partition-row contiguous, which the packet engines move ~2x
            # faster than 1 KB rows (measured).
            def wdma(ka, kb, o):
                a, b = wo(ka, o), wo(kb - 1, o) + ON
                nc.scalar.dma_start(wall[:, a:b], wt[:, a:b])

            def xodma(ka, kb):
                nc.scalar.dma_start(
                    xo_s[:, ka * XK : kb * XK], xo[:, ka * XK : kb * XK]
                )

            nc.scalar.dma_start(xo_s[:, : XK // 2], xo[:, : XK // 2])
            wdma(0, 1, 0)
            nc.scalar.dma_start(xo_s[:, XK // 2 : XK], xo[:, XK // 2 : XK])
            for j in range(4):  # k-steps 1-8: xo singles, W pairs
                xodma(2 * j + 1, 2 * j + 2)
                wdma(2 * j + 1, 2 * j + 3, 0)
                xodma(2 * j + 2, 2 * j + 3)
            for j in range(3):  # k-steps 9-14: xo pairs, W pairs
                wdma(2 * j + 9, 2 * j + 11, 0)
                xodma(2 * j + 9, 2 * j + 11)
            wdma(15, 16, 0)
            xodma(15, 16)
            for o in range(1, NO):
                for k in range(0, 16, 4):
                    wdma(k, k + 4, o)

            def bank(slot, name):
                return pspool.tile([P, ON], fp32, tag=f"ps{slot}", name=name)

            def xsl(k, t):
                a = k * XK + t * P
                return xo_s[:, a : a + P]

            # PE warm-up: garbage matmuls on zeroed scratch open the HAM
            # clock-gate (needs ~3.4us of busy) before real data arrives
            # (~2 us after the first two triggers complete).
            ps_oct = [bank(t, f"ps_a_{t}") for t in range(NOCT)]
            for _ in range(N_WARM):
                nc.tensor.matmul(
                    ps_oct[0][:, :P], scr[:, :P], scr[:, :P], start=True, stop=True
                )

            # ---- early phase: 8-tile interleave, 4 o-sweeps ----
            KSW = 12  # k-outer for k<KSW, t-major for the last 4 k-steps
            for o in range(NO):
                if o > 0:
                    ps_oct = [bank(t, f"ps_{o}_{t}") for t in range(NOCT)]
                for k in range(KSW):
                    for t in range(NOCT):
                        nc.tensor.matmul(
                            ps_oct[t][:],
                            xsl(k, t),
                            wall[:, wo(k, o) : wo(k, o) + ON],
                            start=(k == 0),
                            stop=False,
                        )
                for t in range(NOCT):
                    for k in range(KSW, KT):
                        nc.tensor.matmul(
                            ps_oct[t][:],
                            xsl(k, t),
                            wall[:, wo(k, o) : wo(k, o) + ON],
                            start=False,
                            stop=(k == KT - 1),
                        )
                    oq = oqpool.tile([P, ON], fp32, tag="oq", name=f"oq_{o}_{t}")
                    nc.vector.tensor_copy(oq[:], ps_oct[t][:])
                    nc.sync.dma_start(
                        out[t * P : (t + 1) * P, o * ON : (o + 1) * ON], oq[:]
                    )

            # ---- steady phase: t=8..14, one tile at a time ----
            x_tiles = {}
            for t in (8, 9, 10):
                x_tiles[t] = xpool.tile([P, KT * P], idt, tag="x", name=f"x_{t}")
                nc.scalar.dma_start(x_tiles[t][:], xt[t - NOCT])
            for t in range(NOCT, TT - 1):
                if t not in x_tiles:
                    x_tiles[t] = xpool.tile([P, KT * P], idt, tag="x", name=f"x_{t}")
                    nc.scalar.dma_start(x_tiles[t][:], xt[t - NOCT])
                s = t % 2
                pst = [bank(4 * s + o, f"ps_{t}_{o}") for o in range(NO)]
                for k in range(KT):
                    lhsT = x_tiles[t][:, k * P : (k + 1) * P]
                    for o in range(NO):
                        nc.tensor.matmul(
                            pst[o][:],
                            lhsT,
                            wall[:, wo(k, o) : wo(k, o) + ON],
                            start=(k == 0),
                            stop=(k == KT - 1),
                        )
                o_t = opool.tile([P, D_OUT], fp32, tag="o", name=f"o_{t}")
                for o in range(NO):
                    nc.vector.tensor_copy(o_t[:, o * ON : (o + 1) * ON], pst[o][:])
                nc.sync.dma_start(
                    out[t * P : (t + 1) * P, : D_OUT // 2], o_t[:, : D_OUT // 2]
                )
                nc.sync.dma_start(
                    out[t * P : (t + 1) * P, D_OUT // 2 :], o_t[:, D_OUT // 2 :]
                )

            # ---- tail tile t=15: o-outer so banks complete progressively;
            # the last bank's copy-out is split 2x256 to shorten the tail ----
            t = TT - 1
            x_tiles[t] = xpool.tile([P, KT * P], idt, tag="x", name=f"x_{t}")
            nc.scalar.dma_start(x_tiles[t][:], xt[t - NOCT])
            ps15 = [bank(4 + o, f"ps_15_{o}") for o in range(NO)]
            o_t = opool.tile([P, D_OUT], fp32, tag="o", name="o_15")
            out_eng = [nc.sync, nc.sync, nc.gpsimd]
            for o in range(NO):
                for k in range(KT):
                    nc.tensor.matmul(
                        ps15[o][:],
                        x_tiles[t][:, k * P : (k + 1) * P],
                        wall[:, wo(k, o) : wo(k, o) + ON],
                        start=(k == 0),
                        stop=(k == KT - 1),
                    )
                if o < NO - 1:
                    nc.vector.tensor_copy(o_t[:, o * ON : (o + 1) * ON], ps15[o][:])
                    out_eng[o].dma_start(
                        out[t * P : (t + 1) * P, o * ON : (o + 1) * ON],
                        o_t[:, o * ON : (o + 1) * ON],
                    )
                else:
                    h = ON // 2
                    a = o * ON
                    nc.vector.tensor_copy(o_t[:, a : a + h], ps15[o][:, :h])
                    nc.scalar.dma_start(
                        out[t * P : (t + 1) * P, a : a + h], o_t[:, a : a + h]
                    )
                    nc.vector.tensor_copy(o_t[:, a + h : a + ON], ps15[o][:, h:])
                    nc.gpsimd.dma_start(
                        out[t * P : (t + 1) * P, a + h : a + ON],
                        o_t[:, a + h : a + ON],
                    )

    nc.compile()
    return nc


def _get_nc():
    if IN_DTYPE not in _NC:
        _NC[IN_DTYPE] = _build_nc()
    return _NC[IN_DTYPE]


def _prep_inputs(inputs):
    x = np.ascontiguousarray(np.asarray(inputs["x"], dtype=np.float32))
    W = np.asarray(inputs["W"], dtype=np.float32)
    lora_a = np.asarray(inputs["lora_a"], dtype=np.float32)
    lora_b = np.asarray(inputs["lora_b"], dtype=np.float32)
    scalings = np.asarray(inputs["scalings"], dtype=np.float32)
    idt = _np_in_dtype()

    # Fold LoRA into the transposed effective weight per adapter:
    # Weff.T = W.T + s * A.T @ B.T  -> [d_in, d_out], laid out o-major:
    # wt[p, (o*KT + k)*512 + c] = Weff.T[k*128+p, o*512+c].
    wts = []
    for g in range(N_ADAPTERS):
        weff_t = W.T + scalings[g] * (lora_a[g].T @ lora_b[g].T)
        wts.append(
            np.ascontiguousarray(
                weff_t.reshape(KT, P, NO, ON).transpose(1, 2, 0, 3).astype(idt)
            ).reshape(P, NO * KT * ON)
        )

    in_maps = []
    for c in range(N_CORES):
        xs = x[c * T_LOC : (c + 1) * T_LOC]  # [2048 tok, 2048 d]
        # tiles 0-7, k-major octo layout: [t, j, k, p] -> [p, k, t, j]
        xol = np.ascontiguousarray(
            xs[: NOCT * P].reshape(NOCT, P, KT, P).transpose(3, 2, 0, 1).astype(idt)
        ).reshape(P, KT * NOCT * P)
        # tiles 8-15, tile-major: [t, j, k, p] -> [t, p, k, j]
        xtl = np.ascontiguousarray(
            xs[NOCT * P :]
            .reshape(TT - NOCT, P, KT, P)
            .transpose(0, 3, 2, 1)
            .astype(idt)
        ).reshape(TT - NOCT, P, KT * P)
        in_maps.append(
            {
                "xo": xol,
                "xt": xtl,
                "wt": wts[c * T_LOC // (TOKENS // N_ADAPTERS)],
            }
        )
    return in_maps


def _run(inputs, trace=False, **kwargs):
    from concourse.bass_utils import run_bass_kernel_spmd

    nc = _get_nc()
    in_maps = _prep_inputs(inputs)
    res = run_bass_kernel_spmd(
        nc, in_maps, core_ids=list(range(N_CORES)), trace=trace, **kwargs
    )
    out = np.concatenate([r["out"] for r in res.results], axis=0)
    return out, res


def kernel(**inputs):
    out, _ = _run(inputs, trace=False)
    return out
